# revision 13
# baseline (speedup 1.0000x reference)
"""Bass/Tile TRN2 kernel for nn_Attention (B=32, S=2048, D=1024), 8 cores.

Algorithm (algebraically equal to the reference):
    v[b,:]  = st[b] @ W                  (tiny rank-32 precompute, done host-side)
    c[b]    = st[b] . b                  (host-side)
    score   = (hx . v + c) * (mask + 1e-18)
    e       = exp(score - max); L = sum(e)
    u[b,:]  = e . hx                     (PE matmul, hx in native layout)
    ct      = (u @ W^T)/L + b            (softmax sums to 1)

Data-parallel over batch: each of the 8 cores gets 4 batches; W/b replicated.
hx is streamed from HBM exactly once (32MB/core) -> memory-bound kernel.

Schedule (v3):
  - hx streaming starts immediately (v/c are host-side aux inputs, so the
    DVE score pipeline starts at ~10us instead of ~25us).
  - W streams between batch 1 and batch 2; its PE transposes (for the
    final u @ W^T) overlap batch 2's stream.
  - u accumulation split 12/4 with online max-rescale
    (u = u_A*exp(mA-m) + u_B as one extra PE accumulate), so only 8
    u-matmuls + softmax smalls + the ct projection sit in the post-DMA
    tail.
  - final ct projection batched over all 4 batches ([4,512] matmuls).
"""

import numpy as np
from contextlib import ExitStack

import concourse.bass as bass
import concourse.bacc as bacc
import concourse.bass_isa as bass_isa
import concourse.mybir as mybir
import concourse.tile as tile
from concourse.bass_utils import run_bass_kernel_spmd

B, S, D = 32, 2048, 1024
NCORES = 8
BPC = B // NCORES          # 4 batches per core
P = 128
NT = S // P                # 16 s-tiles of 128 tokens per batch
NH = 8                     # stream each batch's hx in 8 slices
TPH = NT // NH             # 2 s-tiles per slice
SPLIT = 12                 # u-accumulation split point (tiles 0:12 / 12:16)
HSPL = SPLIT // TPH - 1    # slice index after which half A is complete
DCH = D // P               # 8 chunks of 128 along D
HF = 512                   # fp32 moving-operand limit per matmul
WB = 1                     # stream W after this batch's hx

F32 = mybir.dt.float32
F32R = mybir.dt.float32r
BF16 = mybir.dt.bfloat16
AF = mybir.ActivationFunctionType
ALU = mybir.AluOpType
EPS = 1e-18


def build_nc() -> bass.Bass:
    nc = bacc.Bacc("TRN2", target_bir_lowering=False, debug=False)
    v_d = nc.declare_dram_parameter("v", [BPC, D], F32, isOutput=False)
    aux_d = nc.declare_dram_parameter("aux", [2, BPC], F32, isOutput=False)
    hx_d = nc.declare_dram_parameter("hx", [BPC, S, D], F32, isOutput=False)
    hm_d = nc.declare_dram_parameter("hx_mask", [BPC, S], F32, isOutput=False)
    w_d = nc.declare_dram_parameter("W", [D, D], F32, isOutput=False)
    bv_d = nc.declare_dram_parameter("b", [D], F32, isOutput=False)
    id_d = nc.declare_dram_parameter("ident", [P, P], F32, isOutput=False)
    ct_d = nc.declare_dram_parameter("ct", [BPC, D], F32, isOutput=True)

    with tile.TileContext(nc) as tc, ExitStack() as ctx:
        const = ctx.enter_context(tc.tile_pool(name="const", bufs=1))
        wtp = ctx.enter_context(tc.tile_pool(name="wtp", bufs=1))
        wnatp = ctx.enter_context(tc.tile_pool(name="wnatp", bufs=1))
        hxp = ctx.enter_context(tc.tile_pool(name="hxp", bufs=12))
        scrp = ctx.enter_context(tc.tile_pool(name="scrp", bufs=1))
        smp = ctx.enter_context(tc.tile_pool(name="smp", bufs=2))
        vbp = ctx.enter_context(tc.tile_pool(name="vbp", bufs=4))
        psp = ctx.enter_context(tc.tile_pool(name="psp", bufs=2, space="PSUM"))

        # ---- small inputs on the scalar-engine DMA queue ----
        ident = const.tile([P, P], F32, name="ident_sb")
        nc.scalar.dma_start(out=ident[:, :], in_=id_d[:, :])
        ident_r = const.tile([P, P], F32R, name="ident_r")
        nc.scalar.dma_start(out=ident_r[:, :], in_=id_d[:, :].bitcast(F32R))
        v_sb = const.tile([BPC, D], F32, name="v_sb")
        nc.scalar.dma_start(out=v_sb[:, :], in_=v_d[:, :])
        c_row = const.tile([1, BPC], F32, name="c_row")
        nc.scalar.dma_start(out=c_row[:, :], in_=aux_d[0:1, :])
        ones_r = const.tile([1, BPC], F32R, name="ones_r")
        nc.scalar.dma_start(out=ones_r[:, :], in_=aux_d[1:2, :].bitcast(F32R))
        bias_row = const.tile([1, D], F32, name="bias_row")
        nc.scalar.dma_start(out=bias_row[:, :], in_=bv_d[None, :])
        bias_row_r = const.tile([1, D], F32R, name="bias_row_r")
        nc.scalar.dma_start(out=bias_row_r[:, :], in_=bv_d[None, :].bitcast(F32R))
        mask_nat = const.tile([BPC * NT, P], F32, name="mask_nat")
        nc.scalar.dma_start(
            out=mask_nat[:, :], in_=hm_d.rearrange("b (i p) -> (b i) p", p=P)
        )

        # ---- broadcasts (gpsimd) ----
        c_bcast = const.tile([P, BPC], F32, name="c_bcast")
        nc.gpsimd.partition_broadcast(c_bcast[:, :], c_row[0:1, :])
        vb_tiles = {}
        for b in range(BPC):
            vb = vbp.tile([P, D], F32, name=f"vb{b}", tag="vb")
            if b == 0:
                nc.gpsimd.partition_broadcast(vb[:, :], v_sb[0:1, :])
            else:
                v_row = smp.tile([1, D], F32, name=f"v_row{b}", tag="v_row", bufs=1)
                nc.scalar.dma_start(out=v_row[:, :], in_=v_sb[b:b + 1, :])
                nc.gpsimd.partition_broadcast(vb[:, :], v_row[0:1, :])
            vb_tiles[b] = vb

        # ---- mask -> [128, 4*16] (partition = s%128, col = b*16 + s//128) ----
        tpm = psp.tile([P, P], F32, name="tpm", tag="tp")
        nc.tensor.transpose(
            tpm[:, 0:BPC * NT],
            mask_nat[0:BPC * NT, :],
            ident[0:BPC * NT, 0:BPC * NT],
        )
        mask1 = const.tile([P, BPC * NT], F32, name="mask1")
        # mask + (1-mask)*1e-18 == mask + 1e-18 exactly in fp32
        nc.vector.tensor_scalar_add(mask1[:, :], tpm[:, 0:BPC * NT], EPS)

        # ---- W^T tiles (filled mid-stream, see batch loop) ----
        wt_tiles = [
            wtp.tile([P, D], BF16, name=f"wt{i}", tag=f"wt{i}") for i in range(DCH)
        ]
        wn_all = wnatp.tile([P, DCH * D], F32R, name="wn_all", tag="wn")

        def issue_w_block():
            for j in range(DCH):
                wn = wn_all[:, j * D:(j + 1) * D]
                nc.sync.dma_start(
                    out=wn[:, :], in_=w_d[j * P:(j + 1) * P, :].bitcast(F32R)
                )
                for i in range(DCH):
                    tp = psp.tile([P, P], F32R, name=f"tp_w{j}_{i}", tag="tp")
                    nc.tensor.transpose(
                        tp[:, :], wn[:, i * P:(i + 1) * P], ident_r[:, :]
                    )
                    nc.scalar.copy(wt_tiles[i][:, j * P:(j + 1) * P], tp[:, :])

        # ---- per-batch streaming with split-u ----
        ut_tiles = [
            const.tile([P, BPC], BF16, name=f"ut{k}") for k in range(DCH)
        ]
        for b in range(BPC):
            vb = vb_tiles[b]

            score = smp.tile([P, NT], F32, name=f"score{b}", tag="score")
            e_sb = smp.tile([P, NT], F32R, name=f"e{b}", tag="e")
            hx_half = []
            uA_ps = [
                psp.tile([1, HF], F32, name=f"uA_ps{b}_{h}", tag=f"uA{h}", bufs=1)
                for h in range(2)
            ]
            uB_ps = [
                psp.tile([1, HF], F32, name=f"uB_ps{b}_{h}", tag=f"uB{h}", bufs=1)
                for h in range(2)
            ]
            mA = smp.tile([P, 1], F32, name=f"mA_{b}", tag="mA")
            l1A = smp.tile([P, 1], F32, name=f"l1A_{b}", tag="l1A")
            LA = smp.tile([P, 1], F32, name=f"LA_{b}", tag="LA")
            uA_sb = smp.tile([1, D], F32R, name=f"uAsb{b}", tag="uAsb", bufs=1)

            for h in range(NH):
                hxt = hxp.tile([P, TPH * D], F32R, name=f"hx{b}_{h}", tag="hx")
                nc.sync.dma_start(
                    out=hxt[:, :].rearrange("p (i d) -> p i d", d=D),
                    in_=hx_d[b, h * TPH * P:(h + 1) * TPH * P, :].rearrange(
                        "(i p) d -> p i d", p=P
                    ).bitcast(F32R),
                )
                hx_half.append(hxt)
                warm = psp.tile([1, HF], F32, name=f"warm{b}_{h}", tag="tp")
                nc.tensor.matmul(
                    warm[:, :], hxt[:, 0:1], hxt[:, 0:HF], start=True, stop=True,
                )
                for i2 in range(TPH):
                    i = h * TPH + i2
                    scr = scrp.tile([P, D], F32, name=f"scr{b}_{i}", tag="scr")
                    nc.vector.scalar_tensor_tensor(
                        out=scr[:, :],
                        in0=hxt[:, i2 * D:(i2 + 1) * D].bitcast(F32),
                        scalar=1.0,
                        in1=vb[:, :],
                        op0=ALU.mult,
                        op1=ALU.mult,
                        accum_out=score[:, i:i + 1],
                    )

                if h == HSPL:
                    # ---- part A (tiles 0:SPLIT): provisional softmax + u_A ----
                    sA = smp.tile([P, SPLIT], F32, name=f"sA{b}", tag="sA")
                    nc.vector.scalar_tensor_tensor(
                        out=sA[:, :], in0=score[:, 0:SPLIT],
                        scalar=c_bcast[:, b:b + 1],
                        in1=mask1[:, b * NT:b * NT + SPLIT],
                        op0=ALU.add, op1=ALU.mult,
                    )
                    m1 = smp.tile([P, 1], F32, name=f"m1A_{b}", tag="m1")
                    nc.vector.tensor_reduce(
                        m1[:, :], sA[:, :], mybir.AxisListType.X, ALU.max
                    )
                    nc.gpsimd.partition_all_reduce(
                        mA[:, :], m1[:, :], P, bass_isa.ReduceOp.max
                    )
                    negA = smp.tile([P, 1], F32, name=f"negA_{b}", tag="negA")
                    nc.vector.tensor_scalar_mul(negA[:, :], mA[:, :], -1.0)
                    nc.scalar.activation(
                        e_sb[:, 0:SPLIT], sA[:, :], AF.Exp,
                        bias=negA[:, 0:1], scale=1.0, accum_out=l1A[:, 0:1],
                    )
                    nc.gpsimd.partition_all_reduce(
                        LA[:, :], l1A[:, :], P, bass_isa.ReduceOp.add
                    )
                    for i in range(SPLIT):
                        h2, i2 = divmod(i, TPH)
                        for hf in range(2):
                            nc.tensor.matmul(
                                uA_ps[hf][:, :],
                                e_sb[:, i:i + 1],
                                hx_half[h2][:, i2 * D + hf * HF:i2 * D + (hf + 1) * HF],
                                start=(i == 0), stop=(i == SPLIT - 1),
                            )
                    # park u_A in SBUF so part B can rescale-accumulate it
                    for hf in range(2):
                        nc.scalar.copy(uA_sb[:, hf * HF:(hf + 1) * HF],
                                       uA_ps[hf][:, :])

            # ---- part B (tiles SPLIT:NT): final max, rescale-accumulate ----
            sB = smp.tile([P, NT - SPLIT], F32, name=f"sB{b}", tag="sB")
            nc.vector.scalar_tensor_tensor(
                out=sB[:, :], in0=score[:, SPLIT:NT],
                scalar=c_bcast[:, b:b + 1],
                in1=mask1[:, b * NT + SPLIT:(b + 1) * NT],
                op0=ALU.add, op1=ALU.mult,
            )
            m1b = smp.tile([P, 1], F32, name=f"m1B_{b}", tag="m1")
            nc.vector.tensor_reduce(
                m1b[:, :], sB[:, :], mybir.AxisListType.X, ALU.max
            )
            mB = smp.tile([P, 1], F32, name=f"mB_{b}", tag="mB")
            nc.gpsimd.partition_all_reduce(
                mB[:, :], m1b[:, :], P, bass_isa.ReduceOp.max
            )
            mF = smp.tile([P, 1], F32, name=f"mF_{b}", tag="mF")
            nc.vector.scalar_tensor_tensor(
                out=mF[:, :], in0=mA[:, :], scalar=1.0, in1=mB[:, :],
                op0=ALU.mult, op1=ALU.max,
            )
            negF = smp.tile([P, 1], F32, name=f"negF_{b}", tag="negF")
            nc.vector.tensor_scalar_mul(negF[:, :], mF[:, :], -1.0)
            l1B = smp.tile([P, 1], F32, name=f"l1B_{b}", tag="l1B")
            nc.scalar.activation(
                e_sb[:, SPLIT:NT], sB[:, :], AF.Exp,
                bias=negF[:, 0:1], scale=1.0, accum_out=l1B[:, 0:1],
            )
            LB = smp.tile([P, 1], F32, name=f"LB_{b}", tag="LB")
            nc.gpsimd.partition_all_reduce(
                LB[:, :], l1B[:, :], P, bass_isa.ReduceOp.add
            )
            # alpha = exp(mA - mF)   [1,1]  (F32R so it can feed the PE)
            alpha = smp.tile([1, 1], F32R, name=f"alpha_{b}", tag="alpha")
            nc.scalar.activation(
                alpha[:, :], mA[0:1, 0:1], AF.Exp,
                bias=negF[0:1, 0:1], scale=1.0,
            )
            for i in range(SPLIT, NT):
                h2, i2 = divmod(i, TPH)
                for hf in range(2):
                    nc.tensor.matmul(
                        uB_ps[hf][:, :],
                        e_sb[:, i:i + 1],
                        hx_half[h2][:, i2 * D + hf * HF:i2 * D + (hf + 1) * HF],
                        start=(i == SPLIT), stop=False,
                    )
            # rescale-accumulate: uB_ps += alpha * uA  (one PE op per half)
            for hf in range(2):
                nc.tensor.matmul(
                    uB_ps[hf][:, :], alpha[:, :],
                    uA_sb[:, hf * HF:(hf + 1) * HF],
                    start=False, stop=True,
                )
            # L = LA*alpha + LB ; recip
            Lt = smp.tile([1, 1], F32, name=f"L_{b}", tag="Lt")
            nc.vector.scalar_tensor_tensor(
                out=Lt[:, :], in0=LA[0:1, 0:1],
                scalar=alpha[0:1, 0:1].bitcast(F32),
                in1=LB[0:1, 0:1], op0=ALU.mult, op1=ALU.add,
            )
            recip_l = smp.tile([1, 1], F32, name=f"recipl_{b}", tag="recipl")
            nc.vector.reciprocal(recip_l[:, :], Lt[:, :])

            # uhat = (uA*alpha + uB) / L
            uhat = smp.tile([1, D], F32, name=f"uhat{b}", tag="uhat", bufs=1)
            for hf in range(2):
                nc.scalar.mul(
                    uhat[:, hf * HF:(hf + 1) * HF],
                    uB_ps[hf][:, :],
                    mul=recip_l[0:1, 0:1],
                )
            for k in range(DCH):
                tp = psp.tile([P, P], F32, name=f"tp_u{b}_{k}", tag="tp")
                nc.tensor.transpose(
                    tp[:, 0:1], uhat[0:1, k * P:(k + 1) * P], ident[0:1, 0:1]
                )
                nc.scalar.copy(ut_tiles[k][:, b:b + 1], tp[:, 0:1])

            if b == WB:
                # stream W + build W^T tiles while batch WB+1 streams
                issue_w_block()

        # ---- batched final projection: ct = uhatT @ W^T + b, all 4 batches ----
        ct_sb = const.tile([BPC, D], F32, name="ct_sb")
        for hf in range(2):
            ctp = psp.tile([BPC, HF], F32, name=f"ct_ps{hf}", tag=f"uA{hf}", bufs=1)
            for k in range(DCH):
                nc.tensor.matmul(
                    ctp[:, :], ut_tiles[k][:, 0:BPC],
                    wt_tiles[k][:, hf * HF:(hf + 1) * HF],
                    start=(k == 0), stop=False,
                )
            nc.tensor.matmul(
                ctp[:, :], ones_r[:, 0:BPC],
                bias_row_r[:, hf * HF:(hf + 1) * HF],
                start=False, stop=True,
            )
            nc.scalar.copy(ct_sb[:, hf * HF:(hf + 1) * HF], ctp[:, :])
        nc.scalar.dma_start(out=ct_d[:, :], in_=ct_sb[:, :])

    nc.compile()
    return nc


_NC_CACHE = {}


def get_nc() -> bass.Bass:
    if "nc" not in _NC_CACHE:
        _NC_CACHE["nc"] = build_nc()
    return _NC_CACHE["nc"]


def make_in_maps(st, hx, hx_mask, W, b):
    ident = np.eye(P, dtype=np.float32)
    Wf = np.asarray(W, dtype=np.float32)
    bf = np.asarray(b, dtype=np.float32)
    st64 = np.asarray(st, dtype=np.float64)
    v_all = (st64 @ Wf.astype(np.float64)).astype(np.float32)      # [B, D]
    c_all = (st64 * bf.astype(np.float64)).sum(axis=1).astype(np.float32)  # [B]
    in_maps = []
    for i in range(NCORES):
        sl = slice(i * BPC, (i + 1) * BPC)
        aux = np.stack([c_all[sl], np.ones(BPC, dtype=np.float32)]).astype(
            np.float32
        )
        in_maps.append(
            {
                "v": np.ascontiguousarray(v_all[sl]),
                "aux": np.ascontiguousarray(aux),
                "hx": np.ascontiguousarray(hx[sl], dtype=np.float32),
                "hx_mask": np.ascontiguousarray(hx_mask[sl], dtype=np.float32),
                "W": Wf,
                "b": bf,
                "ident": ident,
            }
        )
    return in_maps


def kernel(st, hx, hx_mask, W, b):
    nc = get_nc()
    in_maps = make_in_maps(st, hx, hx_mask, W, b)
    res = run_bass_kernel_spmd(nc, in_maps, list(range(NCORES)))
    out = np.concatenate([res.results[i]["ct"] for i in range(NCORES)], axis=0)
    return out.astype(np.float32)


# revision 19
# speedup vs baseline: 1.3130x; 1.3130x over previous
"""Bass/Tile TRN2 kernel for nn_Attention (B=32, S=2048, D=1024), 8 cores.

Algorithm (algebraically equal to the reference):
    v[b,:]  = st[b] @ W                  (tiny rank-32 precompute, done host-side)
    c[b]    = st[b] . b                  (host-side)
    score   = (hx . v + c) * (mask + 1e-18)
    e       = exp(score - max); L = sum(e)
    u[b,:]  = e . hx                     (PE matmul, hx in native layout)
    ct      = (u @ W^T)/L + b            (softmax sums to 1)

Data-parallel over batch: each of the 8 cores gets 4 batches; W/b replicated.
hx is streamed from HBM exactly once (32MB/core) -> memory-bound kernel.

Schedule (v3):
  - hx streaming starts immediately (v/c are host-side aux inputs, so the
    DVE score pipeline starts at ~10us instead of ~25us).
  - W streams between batch 1 and batch 2; its PE transposes (for the
    final u @ W^T) overlap batch 2's stream.
  - u accumulation split 12/4 with online max-rescale
    (u = u_A*exp(mA-m) + u_B as one extra PE accumulate), so only 8
    u-matmuls + softmax smalls + the ct projection sit in the post-DMA
    tail.
  - final ct projection batched over all 4 batches ([4,512] matmuls).
"""

import numpy as np
from contextlib import ExitStack

import concourse.bass as bass
import concourse.bacc as bacc
import concourse.bass_isa as bass_isa
import concourse.mybir as mybir
import concourse.tile as tile
from concourse.bass_utils import run_bass_kernel_spmd

B, S, D = 32, 2048, 1024
NCORES = 8
BPC = B // NCORES          # 4 batches per core
P = 128
NT = S // P                # 16 s-tiles of 128 tokens per batch
NH = 8                     # stream each batch's hx in 8 slices
TPH = NT // NH             # 2 s-tiles per slice
SPLIT = 12                 # u-accumulation split point (tiles 0:12 / 12:16)
HSPL = SPLIT // TPH - 1    # slice index after which half A is complete
DCH = D // P               # 8 chunks of 128 along D
HF = 512                   # fp32 moving-operand limit per matmul
WB = 2                     # stream W^T after this batch's hx

F32 = mybir.dt.float32
F32R = mybir.dt.float32r
BF16 = mybir.dt.bfloat16
AF = mybir.ActivationFunctionType
ALU = mybir.AluOpType
EPS = 1e-18


def build_nc() -> bass.Bass:
    nc = bacc.Bacc("TRN2", target_bir_lowering=False, debug=False)
    v_d = nc.declare_dram_parameter("v", [BPC, D], F32, isOutput=False)
    aux_d = nc.declare_dram_parameter("aux", [2, BPC], F32, isOutput=False)
    hx_d = nc.declare_dram_parameter("hx", [BPC, S, D], F32, isOutput=False)
    hm_d = nc.declare_dram_parameter("hx_mask", [BPC, S], F32, isOutput=False)
    wt_d = nc.declare_dram_parameter("WT16", [D, D], BF16, isOutput=False)
    bv_d = nc.declare_dram_parameter("b", [D], F32, isOutput=False)
    id_d = nc.declare_dram_parameter("ident", [P, P], F32, isOutput=False)
    ct_d = nc.declare_dram_parameter("ct", [BPC, D], F32, isOutput=True)

    with tile.TileContext(nc) as tc, ExitStack() as ctx:
        const = ctx.enter_context(tc.tile_pool(name="const", bufs=1))
        wtp = ctx.enter_context(tc.tile_pool(name="wtp", bufs=1))
        hxp = ctx.enter_context(tc.tile_pool(name="hxp", bufs=14))
        scrp = ctx.enter_context(tc.tile_pool(name="scrp", bufs=1))
        smp = ctx.enter_context(tc.tile_pool(name="smp", bufs=2))
        vbp = ctx.enter_context(tc.tile_pool(name="vbp", bufs=4))
        psp = ctx.enter_context(tc.tile_pool(name="psp", bufs=2, space="PSUM"))

        # ---- small inputs on the scalar-engine DMA queue ----
        ident = const.tile([P, P], F32, name="ident_sb")
        nc.scalar.dma_start(out=ident[:, :], in_=id_d[:, :])
        v_sb = const.tile([BPC, D], F32, name="v_sb")
        nc.scalar.dma_start(out=v_sb[:, :], in_=v_d[:, :])
        c_row = const.tile([1, BPC], F32, name="c_row")
        nc.scalar.dma_start(out=c_row[:, :], in_=aux_d[0:1, :])
        ones_r = const.tile([1, BPC], F32R, name="ones_r")
        nc.scalar.dma_start(out=ones_r[:, :], in_=aux_d[1:2, :].bitcast(F32R))
        bias_row = const.tile([1, D], F32, name="bias_row")
        nc.scalar.dma_start(out=bias_row[:, :], in_=bv_d[None, :])
        bias_row_r = const.tile([1, D], F32R, name="bias_row_r")
        nc.scalar.dma_start(out=bias_row_r[:, :], in_=bv_d[None, :].bitcast(F32R))
        mask_nat = const.tile([BPC * NT, P], F32, name="mask_nat")
        nc.scalar.dma_start(
            out=mask_nat[:, :], in_=hm_d.rearrange("b (i p) -> (b i) p", p=P)
        )

        # ---- broadcasts (gpsimd) ----
        c_bcast = const.tile([P, BPC], F32, name="c_bcast")
        nc.gpsimd.partition_broadcast(c_bcast[:, :], c_row[0:1, :])
        vb_tiles = {}
        for b in range(BPC):
            vb = vbp.tile([P, D], F32, name=f"vb{b}", tag="vb")
            if b == 0:
                nc.gpsimd.partition_broadcast(vb[:, :], v_sb[0:1, :])
            else:
                v_row = smp.tile([1, D], F32, name=f"v_row{b}", tag="v_row", bufs=1)
                nc.scalar.dma_start(out=v_row[:, :], in_=v_sb[b:b + 1, :])
                nc.gpsimd.partition_broadcast(vb[:, :], v_row[0:1, :])
            vb_tiles[b] = vb

        # ---- mask -> [128, 4*16] (partition = s%128, col = b*16 + s//128) ----
        tpm = psp.tile([P, P], F32, name="tpm", tag="tp")
        nc.tensor.transpose(
            tpm[:, 0:BPC * NT],
            mask_nat[0:BPC * NT, :],
            ident[0:BPC * NT, 0:BPC * NT],
        )
        mask1 = const.tile([P, BPC * NT], F32, name="mask1")
        # mask + (1-mask)*1e-18 == mask + 1e-18 exactly in fp32
        nc.vector.tensor_scalar_add(mask1[:, :], tpm[:, 0:BPC * NT], EPS)

        # ---- W^T tiles: direct bf16 DMA, streamed mid-stream (see batch loop) ----
        wt_tiles = [
            wtp.tile([P, D], BF16, name=f"wt{i}", tag=f"wt{i}") for i in range(DCH)
        ]

        def issue_w_block():
            for i in range(DCH):
                nc.sync.dma_start(
                    out=wt_tiles[i][:, :], in_=wt_d[i * P:(i + 1) * P, :]
                )

        # ---- per-batch streaming with split-u ----
        ut_tiles = [
            const.tile([P, BPC], BF16, name=f"ut{k}") for k in range(DCH)
        ]
        for b in range(BPC):
            vb = vb_tiles[b]

            score = smp.tile([P, NT], F32, name=f"score{b}", tag="score")
            e_sb = smp.tile([P, NT], F32R, name=f"e{b}", tag="e")
            hx_half = []
            uA_ps = [
                psp.tile([1, HF], F32, name=f"uA_ps{b}_{h}", tag=f"uA{h}", bufs=1)
                for h in range(2)
            ]
            uB_ps = [
                psp.tile([1, HF], F32, name=f"uB_ps{b}_{h}", tag=f"uB{h}", bufs=1)
                for h in range(2)
            ]
            mA = smp.tile([P, 1], F32, name=f"mA_{b}", tag="mA")
            l1A = smp.tile([P, 1], F32, name=f"l1A_{b}", tag="l1A")
            LA = smp.tile([P, 1], F32, name=f"LA_{b}", tag="LA")
            uA_sb = smp.tile([1, D], F32R, name=f"uAsb{b}", tag="uAsb", bufs=1)

            for h in range(NH):
                hxt = hxp.tile([P, TPH * D], F32R, name=f"hx{b}_{h}", tag="hx")
                nc.sync.dma_start(
                    out=hxt[:, :].rearrange("p (i d) -> p i d", d=D),
                    in_=hx_d[b, h * TPH * P:(h + 1) * TPH * P, :].rearrange(
                        "(i p) d -> p i d", p=P
                    ).bitcast(F32R),
                )
                hx_half.append(hxt)
                warm = psp.tile([1, HF], F32, name=f"warm{b}_{h}", tag="tp")
                nc.tensor.matmul(
                    warm[:, :], hxt[:, 0:1], hxt[:, 0:HF], start=True, stop=True,
                )
                for i2 in range(TPH):
                    i = h * TPH + i2
                    scr = scrp.tile([P, D], F32, name=f"scr{b}_{i}", tag="scr")
                    nc.vector.scalar_tensor_tensor(
                        out=scr[:, :],
                        in0=hxt[:, i2 * D:(i2 + 1) * D].bitcast(F32),
                        scalar=1.0,
                        in1=vb[:, :],
                        op0=ALU.mult,
                        op1=ALU.mult,
                        accum_out=score[:, i:i + 1],
                    )

                if h == HSPL:
                    # ---- part A (tiles 0:SPLIT): provisional softmax + u_A ----
                    sA = smp.tile([P, SPLIT], F32, name=f"sA{b}", tag="sA")
                    nc.vector.scalar_tensor_tensor(
                        out=sA[:, :], in0=score[:, 0:SPLIT],
                        scalar=c_bcast[:, b:b + 1],
                        in1=mask1[:, b * NT:b * NT + SPLIT],
                        op0=ALU.add, op1=ALU.mult,
                    )
                    m1 = smp.tile([P, 1], F32, name=f"m1A_{b}", tag="m1")
                    nc.vector.tensor_reduce(
                        m1[:, :], sA[:, :], mybir.AxisListType.X, ALU.max
                    )
                    nc.gpsimd.partition_all_reduce(
                        mA[:, :], m1[:, :], P, bass_isa.ReduceOp.max
                    )
                    negA = smp.tile([P, 1], F32, name=f"negA_{b}", tag="negA")
                    nc.vector.tensor_scalar_mul(negA[:, :], mA[:, :], -1.0)
                    nc.scalar.activation(
                        e_sb[:, 0:SPLIT], sA[:, :], AF.Exp,
                        bias=negA[:, 0:1], scale=1.0, accum_out=l1A[:, 0:1],
                    )
                    nc.gpsimd.partition_all_reduce(
                        LA[:, :], l1A[:, :], P, bass_isa.ReduceOp.add
                    )
                    for i in range(SPLIT):
                        h2, i2 = divmod(i, TPH)
                        for hf in range(2):
                            nc.tensor.matmul(
                                uA_ps[hf][:, :],
                                e_sb[:, i:i + 1],
                                hx_half[h2][:, i2 * D + hf * HF:i2 * D + (hf + 1) * HF],
                                start=(i == 0), stop=(i == SPLIT - 1),
                            )
                    # park u_A in SBUF so part B can rescale-accumulate it
                    for hf in range(2):
                        nc.scalar.copy(uA_sb[:, hf * HF:(hf + 1) * HF],
                                       uA_ps[hf][:, :])

            # ---- part B (tiles SPLIT:NT): final max, rescale-accumulate ----
            sB = smp.tile([P, NT - SPLIT], F32, name=f"sB{b}", tag="sB")
            nc.vector.scalar_tensor_tensor(
                out=sB[:, :], in0=score[:, SPLIT:NT],
                scalar=c_bcast[:, b:b + 1],
                in1=mask1[:, b * NT + SPLIT:(b + 1) * NT],
                op0=ALU.add, op1=ALU.mult,
            )
            m1b = smp.tile([P, 1], F32, name=f"m1B_{b}", tag="m1")
            nc.vector.tensor_reduce(
                m1b[:, :], sB[:, :], mybir.AxisListType.X, ALU.max
            )
            mB = smp.tile([P, 1], F32, name=f"mB_{b}", tag="mB")
            nc.gpsimd.partition_all_reduce(
                mB[:, :], m1b[:, :], P, bass_isa.ReduceOp.max
            )
            mF = smp.tile([P, 1], F32, name=f"mF_{b}", tag="mF")
            nc.vector.scalar_tensor_tensor(
                out=mF[:, :], in0=mA[:, :], scalar=1.0, in1=mB[:, :],
                op0=ALU.mult, op1=ALU.max,
            )
            negF = smp.tile([P, 1], F32, name=f"negF_{b}", tag="negF")
            nc.vector.tensor_scalar_mul(negF[:, :], mF[:, :], -1.0)
            l1B = smp.tile([P, 1], F32, name=f"l1B_{b}", tag="l1B")
            nc.scalar.activation(
                e_sb[:, SPLIT:NT], sB[:, :], AF.Exp,
                bias=negF[:, 0:1], scale=1.0, accum_out=l1B[:, 0:1],
            )
            LB = smp.tile([P, 1], F32, name=f"LB_{b}", tag="LB")
            nc.gpsimd.partition_all_reduce(
                LB[:, :], l1B[:, :], P, bass_isa.ReduceOp.add
            )
            # alpha = exp(mA - mF)   [1,1]  (F32R so it can feed the PE)
            alpha = smp.tile([1, 1], F32R, name=f"alpha_{b}", tag="alpha")
            nc.scalar.activation(
                alpha[:, :], mA[0:1, 0:1], AF.Exp,
                bias=negF[0:1, 0:1], scale=1.0,
            )
            for i in range(SPLIT, NT):
                h2, i2 = divmod(i, TPH)
                for hf in range(2):
                    nc.tensor.matmul(
                        uB_ps[hf][:, :],
                        e_sb[:, i:i + 1],
                        hx_half[h2][:, i2 * D + hf * HF:i2 * D + (hf + 1) * HF],
                        start=(i == SPLIT), stop=False,
                    )
            # rescale-accumulate: uB_ps += alpha * uA  (one PE op per half)
            for hf in range(2):
                nc.tensor.matmul(
                    uB_ps[hf][:, :], alpha[:, :],
                    uA_sb[:, hf * HF:(hf + 1) * HF],
                    start=False, stop=True,
                )
            # L = LA*alpha + LB ; recip
            Lt = smp.tile([1, 1], F32, name=f"L_{b}", tag="Lt")
            nc.vector.scalar_tensor_tensor(
                out=Lt[:, :], in0=LA[0:1, 0:1],
                scalar=alpha[0:1, 0:1].bitcast(F32),
                in1=LB[0:1, 0:1], op0=ALU.mult, op1=ALU.add,
            )
            recip_l = smp.tile([1, 1], F32, name=f"recipl_{b}", tag="recipl")
            nc.vector.reciprocal(recip_l[:, :], Lt[:, :])

            # uhat = (uA*alpha + uB) / L
            uhat = smp.tile([1, D], F32, name=f"uhat{b}", tag="uhat", bufs=1)
            for hf in range(2):
                nc.scalar.mul(
                    uhat[:, hf * HF:(hf + 1) * HF],
                    uB_ps[hf][:, :],
                    mul=recip_l[0:1, 0:1],
                )
            for k in range(DCH):
                tp = psp.tile([P, P], F32, name=f"tp_u{b}_{k}", tag="tp")
                nc.tensor.transpose(
                    tp[:, 0:1], uhat[0:1, k * P:(k + 1) * P], ident[0:1, 0:1]
                )
                nc.scalar.copy(ut_tiles[k][:, b:b + 1], tp[:, 0:1])

            if b == WB:
                # stream W + build W^T tiles while batch WB+1 streams
                issue_w_block()

        # ---- batched final projection: ct = uhatT @ W^T + b, all 4 batches ----
        ct_sb = const.tile([BPC, D], F32, name="ct_sb")
        for hf in range(2):
            ctp = psp.tile([BPC, HF], F32, name=f"ct_ps{hf}", tag=f"uA{hf}", bufs=1)
            for k in range(DCH):
                nc.tensor.matmul(
                    ctp[:, :], ut_tiles[k][:, 0:BPC],
                    wt_tiles[k][:, hf * HF:(hf + 1) * HF],
                    start=(k == 0), stop=False,
                )
            nc.tensor.matmul(
                ctp[:, :], ones_r[:, 0:BPC],
                bias_row_r[:, hf * HF:(hf + 1) * HF],
                start=False, stop=True,
            )
            nc.scalar.copy(ct_sb[:, hf * HF:(hf + 1) * HF], ctp[:, :])
        nc.scalar.dma_start(out=ct_d[:, :], in_=ct_sb[:, :])

    nc.compile()
    return nc


_NC_CACHE = {}


def get_nc() -> bass.Bass:
    if "nc" not in _NC_CACHE:
        _NC_CACHE["nc"] = build_nc()
    return _NC_CACHE["nc"]


def make_in_maps(st, hx, hx_mask, W, b):
    from ml_dtypes import bfloat16

    ident = np.eye(P, dtype=np.float32)
    Wf = np.asarray(W, dtype=np.float32)
    bf = np.asarray(b, dtype=np.float32)
    # device consumes only the bf16 rounding of W^T (ct projection weights)
    WT16 = np.ascontiguousarray(Wf.T).astype(bfloat16)
    st64 = np.asarray(st, dtype=np.float64)
    v_all = (st64 @ Wf.astype(np.float64)).astype(np.float32)      # [B, D]
    c_all = (st64 * bf.astype(np.float64)).sum(axis=1).astype(np.float32)  # [B]
    in_maps = []
    for i in range(NCORES):
        sl = slice(i * BPC, (i + 1) * BPC)
        aux = np.stack([c_all[sl], np.ones(BPC, dtype=np.float32)]).astype(
            np.float32
        )
        in_maps.append(
            {
                "v": np.ascontiguousarray(v_all[sl]),
                "aux": np.ascontiguousarray(aux),
                "hx": np.ascontiguousarray(hx[sl], dtype=np.float32),
                "hx_mask": np.ascontiguousarray(hx_mask[sl], dtype=np.float32),
                "WT16": WT16,
                "b": bf,
                "ident": ident,
            }
        )
    return in_maps


def kernel(st, hx, hx_mask, W, b):
    nc = get_nc()
    in_maps = make_in_maps(st, hx, hx_mask, W, b)
    res = run_bass_kernel_spmd(nc, in_maps, list(range(NCORES)))
    out = np.concatenate([res.results[i]["ct"] for i in range(NCORES)], axis=0)
    return out.astype(np.float32)
